# revision 40
# baseline (speedup 1.0000x reference)
"""Trainium2 Bass kernel for nn_CrossAttention (B=4, Lq=Lk=E=1024, H=16).

Sharding: data-parallel over 8 cores; core c handles batch c//2, query rows
(c%2)*512 ... +512. Heads stay local to a core, so softmax + head-mean need
no collectives. Each core computes a [512,1024] slice of attn_output and
attn_scores.

Per-core pipeline (bf16 matmul path, fp16 softmax tiles — fp8 fails the
2e-2 accuracy gate, measured):
  rmsnorm on bf16 rows (DVE fused square+rowsum, ACT sqrt, DVE recip,
  scale split between ACT and DVE) -> PE transpose of xn to [e, row]
  -> q/k projections in 128-col chunks (o-tiles 0,1 inline with phase A
     so attention starts as soon as the rmsnorm pipeline drains)
  -> per head h: logits = qT_h.T @ kT_h (bf16, PE) -> ACT Exp with
     scale=1/sqrt(hd) -> fp16 Et + fused rowsum -> DVE recip ->
     even heads: fp16 diag matmul accumulates w_h*E_h/H into PSUM scores
     (deferred one head so PE never gates the next exp);
     odd heads: gpsimd scalar_tensor_tensor accumulates w_h*E_h into an
     fp16 SBUF accumulator on the otherwise-idle Pool engine
  -> scores = PSUM + acc/H merged by DVE -> bf16 -> PE transpose
  -> attn_output = scT.T @ V (bf16).
"""

import numpy as np
from contextlib import ExitStack

B, LQ, LK, E = 4, 1024, 1024, 1024
H = 16
HD = E // H  # 64
N_CORES = 8
QROWS = LQ // 2  # 512 rows of q per core
EPS = 1.1920929e-07

_CACHE = {}


def _build_program():
    import concourse.bass as bass
    import concourse.tile as tile
    from concourse import bacc, mybir

    f32 = mybir.dt.float32
    bf16 = mybir.dt.bfloat16
    f16 = mybir.dt.float16
    Alu = mybir.AluOpType
    Act = mybir.ActivationFunctionType

    nc = bacc.Bacc("TRN2", target_bir_lowering=False, debug=False,
                   num_devices=N_CORES)

    xq = nc.dram_tensor("xq", [QROWS, E], bf16, kind="ExternalInput").ap()
    xk = nc.dram_tensor("xk", [LK, E], bf16, kind="ExternalInput").ap()
    vv = nc.dram_tensor("vv", [LK, E], bf16, kind="ExternalInput").ap()
    wqt = nc.dram_tensor("wqt", [E, E], bf16, kind="ExternalInput").ap()
    wkt = nc.dram_tensor("wkt", [E, E], bf16, kind="ExternalInput").ap()
    bq = nc.dram_tensor("bq", [128, 8], f32, kind="ExternalInput").ap()
    bk = nc.dram_tensor("bk", [128, 8], f32, kind="ExternalInput").ap()
    ident = nc.dram_tensor("ident", [128, 128], bf16, kind="ExternalInput").ap()

    out = nc.dram_tensor("out", [QROWS, E], bf16, kind="ExternalOutput").ap()
    sc = nc.dram_tensor("sc", [QROWS, LK], bf16, kind="ExternalOutput").ap()

    with tile.TileContext(nc) as tc, ExitStack() as ctx:
        const_pool = ctx.enter_context(tc.tile_pool(name="const", bufs=1))
        id_sb = const_pool.tile([128, 128], bf16)
        nc.sync.dma_start(id_sb[:], ident[:])
        id16 = const_pool.tile([128, 128], f16, name="id16")
        nc.vector.tensor_copy(id16[:], id_sb[:])
        eps_sb = const_pool.tile([128, 1], f32, name="eps_sb")
        nc.vector.memset(eps_sb[:], EPS)
        bq_sb = const_pool.tile([128, 8], f32, name="bq_sb")
        bk_sb = const_pool.tile([128, 8], f32, name="bk_sb")
        nc.sync.dma_start(bq_sb[:], bq[:])
        nc.sync.dma_start(bk_sb[:], bk[:])

        # big persistent SBUF tensors
        big_pool = ctx.enter_context(tc.tile_pool(name="big", bufs=1))
        xk_sb = big_pool.tile([128, 8, E], bf16, name="xk_sb")  # [row%128, row//128, e]
        xq_sb = big_pool.tile([128, 4, E], bf16, name="xq_sb")
        xkT = big_pool.tile([128, 8, LK], bf16, name="xkT")     # [e-part, e-tile, k-row]
        xqT = big_pool.tile([128, 8, QROWS], bf16, name="xqT")  # [e-part, e-tile, q-row]
        kT = big_pool.tile([128, 8, LK], bf16, name="kT")       # [feat-part, o, k-row]
        qT = big_pool.tile([128, 8, QROWS], bf16, name="qT")
        v_sb = big_pool.tile([128, 8, E], bf16, name="v_sb")    # [j-part, j-tile, d]
        wk_sb = big_pool.tile([128, 8, E], bf16, name="wk_sb")  # [e-part, e-tile, o]
        wq_sb = big_pool.tile([128, 8, E], bf16, name="wq_sb")

        stats = ctx.enter_context(tc.tile_pool(name="stats", bufs=12))

        # PSUM (8 banks), LIFO pool nesting:
        #   sp(2) [ tpA(2)+ppA(2) {phase A} | lg(4) [ ppB(2) {it0+proj}
        #   | tpF(2) {finales} ] ]
        sp_pool = ctx.enter_context(tc.tile_pool(name="sp", bufs=1, space="PSUM"))
        pools = {}

        # ---------------- loads: few big DMAs, priority order ---------------
        def load_tiled(dst, src_dram, t0, t1):
            src = src_dram[t0 * 128:t1 * 128, :].rearrange(
                "(t p) e -> p t e", p=128)
            nc.sync.dma_start(dst[:, t0:t1, :], src)

        load_tiled(xk_sb, xk, 0, 2)
        load_tiled(xq_sb, xq, 0, 2)
        load_tiled(xk_sb, xk, 2, 4)
        load_tiled(wk_sb, wkt, 0, 4)
        load_tiled(xk_sb, xk, 4, 6)
        load_tiled(wk_sb, wkt, 4, 8)
        load_tiled(xk_sb, xk, 6, 8)
        load_tiled(xq_sb, xq, 2, 4)
        load_tiled(wq_sb, wqt, 0, 4)
        load_tiled(wq_sb, wqt, 4, 8)

        pp_k = {}
        pp_q = {}

        def proj_chunk(o, which, t):
            # accumulate projection output columns t*128..t*128+128 for o-tile
            if which == "k":
                w_sb, xT, pp_map = wk_sb, xkT, pp_k
            else:
                w_sb, xT, pp_map = wq_sb, xqT, pp_q
            half, tl = divmod(t, 4)
            if (o, half) not in pp_map:
                pp_map[(o, half)] = pools["pp"].tile(
                    [128, 512], f32, tag="pp", name=f"pp_{which}{o}_{half}")
            pk = pp_map[(o, half)]
            cols = slice(tl * 128, tl * 128 + 128)
            for e in range(8):
                nc.tensor.matmul(
                    pk[:, cols], lhsT=w_sb[:, e, o * 128:(o + 1) * 128],
                    rhs=xT[:, e, t * 128:(t + 1) * 128],
                    start=(e == 0), stop=(e == 7),
                )

        def proj_evac(o, which, half):
            if which == "k":
                pp_map, dst, b_sb = pp_k, kT, bk_sb
            else:
                pp_map, dst, b_sb = pp_q, qT, bq_sb
            pk = pp_map.pop((o, half))
            cols = slice(half * 512, half * 512 + 512)
            nc.vector.tensor_scalar(
                out=dst[:, o, cols], in0=pk[:],
                scalar1=b_sb[:, o:o + 1], scalar2=None, op0=Alu.add,
            )

        def proj_o(o, which):
            ntiles = 8 if which == "k" else 4
            for t in range(ntiles):
                proj_chunk(o, which, t)
                if t % 4 == 3:
                    proj_evac(o, which, t // 4)

        # ---------------- phase A: rmsnorm + transpose -----------------------
        with ExitStack() as actx:
            tpA = actx.enter_context(tc.tile_pool(name="tpA", bufs=2, space="PSUM"))
            pools["pp"] = actx.enter_context(
                tc.tile_pool(name="ppA", bufs=2, space="PSUM"))
            sqscr = actx.enter_context(tc.tile_pool(name="sqscr", bufs=2))
            xn_pool = actx.enter_context(tc.tile_pool(name="xn", bufs=2))

            def norm_transpose(x_sb, t, xT_tile, n):
                # tensor_tensor_reduce wedges the device on HW; use the ACT
                # Square+accum path instead (ACT is idle during phase A)
                ssq = stats.tile([128, 1], f32, tag="ssq", name=f"ssq{n}")
                sq = sqscr.tile([128, E], bf16, tag="sq")
                nc.scalar.activation(sq[:], x_sb[:, t, :], Act.Square,
                                     accum_out=ssq[:])
                s = stats.tile([128, 1], f32, tag="s", name=f"s{n}")
                nc.scalar.activation(s[:], ssq[:], Act.Sqrt,
                                     bias=eps_sb[:], scale=1.0 / E)
                inv = stats.tile([128, 1], f32, tag="inv", name=f"inv{n}")
                nc.vector.reciprocal(inv[:], s[:])
                xn = xn_pool.tile([128, E], bf16, tag="xn")
                if n % 2 == 0:
                    # ACT is idle during phase A; do half the scales there
                    nc.scalar.activation(xn[:], x_sb[:, t, :], Act.Copy,
                                         scale=inv[:])
                else:
                    nc.vector.tensor_scalar(
                        out=xn[:], in0=x_sb[:, t, :], scalar1=inv[:],
                        scalar2=None, op0=Alu.mult,
                    )
                tp = tpA.tile([128, 8, 128], bf16, tag="tp")
                for e in range(8):
                    nc.tensor.transpose(
                        tp[:, e, :], xn[:, e * 128:(e + 1) * 128], id_sb[:])
                nc.vector.tensor_copy(
                    xT_tile[:, :, t * 128:(t + 1) * 128], tp[:])

            seq = [("k", 0), ("k", 1), ("q", 0), ("k", 2), ("k", 3), ("q", 1),
                   ("k", 4), ("k", 5), ("q", 2), ("k", 6), ("k", 7), ("q", 3)]
            for n, (which, t) in enumerate(seq):
                if which == "k":
                    norm_transpose(xk_sb, t, xkT, n)
                else:
                    norm_transpose(xq_sb, t, xqT, n)
                # keep o0/o1 projections flowing behind the transposes
                for o in (0, 1):
                    proj_chunk(o, which, t)
                    if t % 4 == 3:
                        proj_evac(o, which, t // 4)

        # ---------------- attention -----------------------------------------
        lg_pool = ctx.enter_context(tc.tile_pool(name="lg", bufs=2, space="PSUM"))
        et_pool = ctx.enter_context(tc.tile_pool(name="et", bufs=4))
        dg_pool = ctx.enter_context(tc.tile_pool(name="dg", bufs=3))
        acc_pool = ctx.enter_context(tc.tile_pool(name="acc", bufs=3))
        scs_pool = ctx.enter_context(tc.tile_pool(name="scs", bufs=2))
        scT_pool = ctx.enter_context(tc.tile_pool(name="scT", bufs=2))
        osb_pool = ctx.enter_context(tc.tile_pool(name="osb", bufs=3))

        sp_tiles = {}
        acc_tiles = {}
        pend_hsum = []

        def flush_hsum():
            # deferred PE head-sum so PE never blocks the next exp's logits;
            # the sp tile is created here (not in attn_head) so the bufs=1
            # slot is only requested after the previous finale released it
            while pend_hsum:
                it, dg, Et, first, last = pend_hsum.pop(0)
                if it not in sp_tiles:
                    sp_tiles[it] = sp_pool.tile([128, LK], f32, tag="sp",
                                                name=f"sp{it}")
                sp = sp_tiles[it]
                for half in range(2):
                    cols = slice(half * 512, half * 512 + 512)
                    nc.tensor.matmul(
                        sp[:, cols], lhsT=dg[:], rhs=Et[:, cols],
                        start=first, stop=last,
                    )

        def attn_head(it, h):
            o, po = h // 2, (h % 2) * 64
            icols = slice(it * 128, (it + 1) * 128)
            lg = lg_pool.tile([128, LK], f32, tag="lg")
            for half in range(2):
                cols = slice(half * 512, half * 512 + 512)
                nc.tensor.matmul(
                    lg[:, cols],
                    lhsT=qT[po:po + 64, o, icols],
                    rhs=kT[po:po + 64, o, cols],
                    start=True, stop=True,
                )
            flush_hsum()
            Et = et_pool.tile([128, LK], f16, tag="et")
            rs = stats.tile([128, 1], f32, tag="rs")
            nc.scalar.activation(Et[:], lg[:], Act.Exp,
                                 scale=1.0 / np.sqrt(HD), accum_out=rs[:])
            w = stats.tile([128, 1], f32, tag="w")
            nc.vector.reciprocal(w[:], rs[:])
            if h % 2 == 0:
                # even heads: fp16 diag matmul into PSUM scores (on PE)
                dg = dg_pool.tile([128, 128], f16, tag="dg")
                nc.vector.tensor_scalar(
                    out=dg[:], in0=id16[:], scalar1=w[:], scalar2=1.0 / H,
                    op0=Alu.mult, op1=Alu.mult,
                )
                pend_hsum.append((it, dg, Et, h == 0, h == H - 2))
                if h == H - 2:
                    flush_hsum()
            else:
                # odd heads: accumulate w_h*E_h on DVE (fp16, 2x mode)
                prev = acc_tiles.get(it)
                acc = acc_pool.tile([128, LK], f16, tag="acc")
                if prev is None:
                    nc.vector.tensor_scalar(
                        out=acc[:], in0=Et[:], scalar1=w[:], scalar2=None,
                        op0=Alu.mult,
                    )
                else:
                    nc.vector.scalar_tensor_tensor(
                        out=acc[:], in0=Et[:], scalar=w[:], in1=prev[:],
                        op0=Alu.mult, op1=Alu.add,
                    )
                acc_tiles[it] = acc

        fin_state = {}

        def finale_part1(it):
            icols = slice(it * 128, (it + 1) * 128)
            sp = sp_tiles.pop(it)
            acc = acc_tiles.pop(it)
            scs = scs_pool.tile([128, LK], bf16, tag="scs")
            nc.vector.scalar_tensor_tensor(
                out=scs[:], in0=acc[:], scalar=1.0 / H, in1=sp[:],
                op0=Alu.mult, op1=Alu.add,
            )
            nc.sync.dma_start(sc[icols, :], scs[:])
            # transpose scores on the DMA xbar instead of PE
            scT = scT_pool.tile([128, 8, 128], bf16, tag="scT")
            nc.sync.dma_start_transpose(scT[:], scs[:])
            fin_state[it] = {"scT": scT, "ov": None}

        def emit_sv(it, jj):
            # one j-chunk of the attn_output matmul; jj in 0..15
            st = fin_state[it]
            icols = slice(it * 128, (it + 1) * 128)
            half, j = divmod(jj, 8)
            cols = slice(half * 512, half * 512 + 512)
            if j == 0:
                st["ov"] = tpF_pool.tile([128, 512], f32, tag="ov",
                                         name=f"ov{it}_{half}")
            nc.tensor.matmul(
                st["ov"][:], lhsT=st["scT"][:, j, :], rhs=v_sb[:, j, cols],
                start=(j == 0), stop=(j == 7),
            )
            if j == 7:
                osb = osb_pool.tile([128, 512], bf16, tag="osb")
                nc.vector.tensor_copy(osb[:], st["ov"][:])
                nc.sync.dma_start(out[icols, cols], osb[:])
                if half == 1:
                    fin_state.pop(it)

        def finale(it):
            finale_part1(it)
            for jj in range(16):
                emit_sv(it, jj)

        # it0 interleaved with remaining projections
        with ExitStack() as bctx:
            pools["pp"] = bctx.enter_context(
                tc.tile_pool(name="ppB", bufs=2, space="PSUM"))
            for p in range(8):
                attn_head(0, 2 * p)
                attn_head(0, 2 * p + 1)
                if p + 2 <= 7:
                    proj_o(p + 2, "k")
                    proj_o(p + 2, "q")

        load_tiled(v_sb, vv, 0, 4)
        load_tiled(v_sb, vv, 4, 8)

        with ExitStack() as fctx:
            tpF_pool = fctx.enter_context(
                tc.tile_pool(name="tpF", bufs=2, space="PSUM"))
            # spread each finale's attn_output matmuls across the next
            # block's heads so PE work never stalls the ACT exp stream
            for it in range(1, 4):
                attn_head(it, 0)
                finale_part1(it - 1)
                for h in range(1, H):
                    attn_head(it, h)
                    emit_sv(it - 1, h - 1)
                emit_sv(it - 1, 15)
            finale(3)

    nc.compile()
    return nc


def _get_program():
    if "nc" not in _CACHE:
        _CACHE["nc"] = _build_program()
    return _CACHE["nc"]


def kernel(query, key, value, gq, gk, Wq, bq, Wk, bk):
    import ml_dtypes
    from concourse.bass_utils import run_bass_kernel_spmd

    nc = _get_program()
    bf16 = ml_dtypes.bfloat16

    wqt = np.ascontiguousarray((np.asarray(Wq) * np.asarray(gq)[None, :]).T
                               ).astype(bf16)
    wkt = np.ascontiguousarray((np.asarray(Wk) * np.asarray(gk)[None, :]).T
                               ).astype(bf16)
    bq2 = np.ascontiguousarray(np.asarray(bq, dtype=np.float32).reshape(8, 128).T)
    bk2 = np.ascontiguousarray(np.asarray(bk, dtype=np.float32).reshape(8, 128).T)
    ident = np.eye(128, dtype=np.float32).astype(bf16)

    q_b = np.asarray(query).astype(bf16)
    k_b = np.asarray(key).astype(bf16)
    v_b = np.asarray(value).astype(bf16)

    in_maps = []
    for c in range(N_CORES):
        b, half = divmod(c, 2)
        i0 = half * QROWS
        in_maps.append({
            "xq": np.ascontiguousarray(q_b[b, i0: i0 + QROWS]),
            "xk": np.ascontiguousarray(k_b[b]),
            "vv": np.ascontiguousarray(v_b[b]),
            "wqt": wqt, "wkt": wkt, "bq": bq2, "bk": bk2, "ident": ident,
        })

    res = run_bass_kernel_spmd(nc, in_maps, list(range(N_CORES)))

    attn_output = np.empty((B, LQ, E), dtype=np.float32)
    attn_scores = np.empty((B, LQ, LK), dtype=np.float32)
    for c in range(N_CORES):
        b, half = divmod(c, 2)
        i0 = half * QROWS
        attn_output[b, i0: i0 + QROWS] = res.results[c]["out"].astype(np.float32)
        attn_scores[b, i0: i0 + QROWS] = res.results[c]["sc"].astype(np.float32)
    return attn_output, attn_scores


# revision 53
# speedup vs baseline: 1.2005x; 1.2005x over previous
"""Trainium2 Bass kernel for nn_CrossAttention (B=4, Lq=Lk=E=1024, H=16).

Sharding: data-parallel over 8 cores; core c handles batch c//2, query rows
(c%2)*512 ... +512. Heads stay local to a core, so softmax + head-mean need
no collectives. Each core computes a [512,1024] slice of attn_output and
attn_scores.

Per-core pipeline (bf16 matmul path, fp16 softmax tiles — fp8 fails the
2e-2 accuracy gate, measured):
  rmsnorm on bf16 rows (DVE fused square+rowsum, ACT sqrt, DVE recip,
  scale split between ACT and DVE) -> PE transpose of xn to [e, row]
  -> q/k projections in 128-col chunks (o-tiles 0,1 inline with phase A
     so attention starts as soon as the rmsnorm pipeline drains)
  -> per head h: logits = qT_h.T @ kT_h (bf16, PE) -> ACT Exp with
     scale=1/sqrt(hd) -> fp16 Et + fused rowsum -> DVE recip ->
     even heads: fp16 diag matmul accumulates w_h*E_h/H into PSUM scores
     (deferred one head so PE never gates the next exp);
     odd heads: gpsimd scalar_tensor_tensor accumulates w_h*E_h into an
     fp16 SBUF accumulator on the otherwise-idle Pool engine
  -> scores = PSUM + acc/H merged by DVE -> bf16 -> PE transpose
  -> attn_output = scT.T @ V (bf16).
"""

import numpy as np
from contextlib import ExitStack

B, LQ, LK, E = 4, 1024, 1024, 1024
H = 16
HD = E // H  # 64
N_CORES = 8
QROWS = LQ // 2  # 512 rows of q per core
EPS = 1.1920929e-07

_CACHE = {}


def _build_program():
    import concourse.bass as bass
    import concourse.tile as tile
    from concourse import bacc, mybir

    f32 = mybir.dt.float32
    bf16 = mybir.dt.bfloat16
    f16 = mybir.dt.float16
    Alu = mybir.AluOpType
    Act = mybir.ActivationFunctionType

    nc = bacc.Bacc("TRN2", target_bir_lowering=False, debug=False,
                   num_devices=N_CORES)

    xq = nc.dram_tensor("xq", [QROWS, E], bf16, kind="ExternalInput").ap()
    xk = nc.dram_tensor("xk", [LK, E], bf16, kind="ExternalInput").ap()
    vv = nc.dram_tensor("vv", [LK, E], bf16, kind="ExternalInput").ap()
    wqt = nc.dram_tensor("wqt", [E, E], bf16, kind="ExternalInput").ap()
    wkt = nc.dram_tensor("wkt", [E, E], bf16, kind="ExternalInput").ap()
    bq = nc.dram_tensor("bq", [128, 8], f32, kind="ExternalInput").ap()
    bk = nc.dram_tensor("bk", [128, 8], f32, kind="ExternalInput").ap()
    ident = nc.dram_tensor("ident", [128, 128], bf16, kind="ExternalInput").ap()

    out = nc.dram_tensor("out", [QROWS, E], bf16, kind="ExternalOutput").ap()
    sc = nc.dram_tensor("sc", [QROWS, LK], bf16, kind="ExternalOutput").ap()

    with tile.TileContext(nc) as tc, ExitStack() as ctx:
        const_pool = ctx.enter_context(tc.tile_pool(name="const", bufs=1))
        id_sb = const_pool.tile([128, 128], bf16)
        nc.sync.dma_start(id_sb[:], ident[:])
        id16 = const_pool.tile([128, 128], f16, name="id16")
        nc.vector.tensor_copy(id16[:], id_sb[:])
        eps_sb = const_pool.tile([128, 1], f32, name="eps_sb")
        nc.vector.memset(eps_sb[:], EPS)
        bq_sb = const_pool.tile([128, 8], f32, name="bq_sb")
        bk_sb = const_pool.tile([128, 8], f32, name="bk_sb")
        nc.sync.dma_start(bq_sb[:], bq[:])
        nc.sync.dma_start(bk_sb[:], bk[:])

        # big persistent SBUF tensors
        big_pool = ctx.enter_context(tc.tile_pool(name="big", bufs=1))
        xk_sb = big_pool.tile([128, 8, E], bf16, name="xk_sb")  # [row%128, row//128, e]
        xq_sb = big_pool.tile([128, 4, E], bf16, name="xq_sb")
        xkT = big_pool.tile([128, 8, LK], bf16, name="xkT")     # [e-part, e-tile, k-row]
        xqT = big_pool.tile([128, 8, QROWS], bf16, name="xqT")  # [e-part, e-tile, q-row]
        kT = big_pool.tile([128, 8, LK], bf16, name="kT")       # [feat-part, o, k-row]
        qT = big_pool.tile([128, 8, QROWS], bf16, name="qT")
        v_sb = big_pool.tile([128, 8, E], bf16, name="v_sb")    # [j-part, j-tile, d]
        wk_sb = big_pool.tile([128, 8, E], bf16, name="wk_sb")  # [e-part, e-tile, o]
        wq_sb = big_pool.tile([128, 8, E], bf16, name="wq_sb")

        stats = ctx.enter_context(tc.tile_pool(name="stats", bufs=12))

        # PSUM (8 banks), LIFO pool nesting:
        #   sp(2) [ tpA(2)+ppA(2) {phase A} | lg(4) [ ppB(2) {it0+proj}
        #   | tpF(2) {finales} ] ]
        sp_pool = ctx.enter_context(tc.tile_pool(name="sp", bufs=1, space="PSUM"))
        pools = {}

        # ---------------- loads: few big DMAs, priority order ---------------
        def load_tiled(dst, src_dram, t0, t1):
            src = src_dram[t0 * 128:t1 * 128, :].rearrange(
                "(t p) e -> p t e", p=128)
            nc.sync.dma_start(dst[:, t0:t1, :], src)

        load_tiled(xk_sb, xk, 0, 2)
        load_tiled(xq_sb, xq, 0, 2)
        load_tiled(wk_sb, wkt, 0, 4)
        load_tiled(xk_sb, xk, 2, 4)
        load_tiled(wk_sb, wkt, 4, 8)
        load_tiled(xk_sb, xk, 4, 6)
        load_tiled(wq_sb, wqt, 0, 4)
        load_tiled(xk_sb, xk, 6, 8)
        load_tiled(xq_sb, xq, 2, 4)
        load_tiled(wq_sb, wqt, 4, 8)

        pp_k = {}
        pp_q = {}

        def proj_chunk(o, which, t):
            # accumulate projection output columns t*128..t*128+128 for o-tile
            if which == "k":
                w_sb, xT, pp_map = wk_sb, xkT, pp_k
            else:
                w_sb, xT, pp_map = wq_sb, xqT, pp_q
            half, tl = divmod(t, 4)
            if (o, half) not in pp_map:
                pp_map[(o, half)] = pools["pp"].tile(
                    [128, 512], f32, tag="pp", name=f"pp_{which}{o}_{half}")
            pk = pp_map[(o, half)]
            cols = slice(tl * 128, tl * 128 + 128)
            for e in range(8):
                nc.tensor.matmul(
                    pk[:, cols], lhsT=w_sb[:, e, o * 128:(o + 1) * 128],
                    rhs=xT[:, e, t * 128:(t + 1) * 128],
                    start=(e == 0), stop=(e == 7),
                )

        def proj_evac(o, which, half):
            if which == "k":
                pp_map, dst, b_sb = pp_k, kT, bk_sb
            else:
                pp_map, dst, b_sb = pp_q, qT, bq_sb
            pk = pp_map.pop((o, half))
            cols = slice(half * 512, half * 512 + 512)
            nc.vector.tensor_scalar(
                out=dst[:, o, cols], in0=pk[:],
                scalar1=b_sb[:, o:o + 1], scalar2=None, op0=Alu.add,
            )

        def proj_o(o, which):
            ntiles = 8 if which == "k" else 4
            for t in range(ntiles):
                proj_chunk(o, which, t)
                if t % 4 == 3:
                    proj_evac(o, which, t // 4)

        # attention SBUF pools created BEFORE phase A so they don't recycle
        # phase-A scratch addresses (recycling adds WAW deps on the slow
        # Pool xn chain, which delayed the first exp by ~17us)
        et_pool = ctx.enter_context(tc.tile_pool(name="et", bufs=6))
        dg_pool = ctx.enter_context(tc.tile_pool(name="dg", bufs=3))
        acc_pool = ctx.enter_context(tc.tile_pool(name="acc", bufs=6))
        scs_pool = ctx.enter_context(tc.tile_pool(name="scs", bufs=2))
        scT_pool = ctx.enter_context(tc.tile_pool(name="scT", bufs=2))
        osb_pool = ctx.enter_context(tc.tile_pool(name="osb", bufs=3))

        # ---------------- phase A: rmsnorm + transpose -----------------------
        with ExitStack() as actx:
            tpA = actx.enter_context(tc.tile_pool(name="tpA", bufs=2, space="PSUM"))
            pools["pp"] = actx.enter_context(
                tc.tile_pool(name="ppA", bufs=2, space="PSUM"))
            sqscr = actx.enter_context(tc.tile_pool(name="sqscr", bufs=2))
            xn_pool = actx.enter_context(tc.tile_pool(name="xn", bufs=2))

            def norm_transpose(x_sb, t, xT_tile, n, which):
                # tensor_tensor_reduce wedges the device on HW; use the ACT
                # Square+accum path instead (ACT is idle during phase A)
                ssq = stats.tile([128, 1], f32, tag="ssq", name=f"ssq{n}")
                sq = sqscr.tile([128, E], bf16, tag="sq")
                nc.scalar.activation(sq[:], x_sb[:, t, :], Act.Square,
                                     accum_out=ssq[:])
                s = stats.tile([128, 1], f32, tag="s", name=f"s{n}")
                nc.scalar.activation(s[:], ssq[:], Act.Sqrt,
                                     bias=eps_sb[:], scale=1.0 / E)
                inv = stats.tile([128, 1], f32, tag="inv", name=f"inv{n}")
                nc.vector.reciprocal(inv[:], s[:])
                xn = xn_pool.tile([128, E], bf16, tag="xn")
                if n % 2 == 0:
                    nc.scalar.activation(xn[:], x_sb[:, t, :], Act.Copy,
                                         scale=inv[:])
                else:
                    nc.vector.tensor_scalar(
                        out=xn[:], in0=x_sb[:, t, :], scalar1=inv[:],
                        scalar2=None, op0=Alu.mult,
                    )
                tp = tpA.tile([128, 8, 128], bf16, tag="tp")
                for e in range(8):
                    nc.tensor.transpose(
                        tp[:, e, :], xn[:, e * 128:(e + 1) * 128], id_sb[:])
                nc.vector.tensor_copy(
                    xT_tile[:, :, t * 128:(t + 1) * 128], tp[:])

            seq = [("k", 0), ("k", 1), ("q", 0), ("k", 2), ("k", 3), ("q", 1),
                   ("k", 4), ("k", 5), ("q", 2), ("k", 6), ("k", 7), ("q", 3)]
            for n, (which, t) in enumerate(seq):
                if which == "k":
                    norm_transpose(xk_sb, t, xkT, n, which)
                else:
                    norm_transpose(xq_sb, t, xqT, n, which)
                # keep o0/o1 projections flowing behind the transposes
                for o in (0, 1):
                    proj_chunk(o, which, t)
                    if t % 4 == 3:
                        proj_evac(o, which, t // 4)

        # ---------------- attention -----------------------------------------
        lg_pool = ctx.enter_context(tc.tile_pool(name="lg", bufs=2, space="PSUM"))

        sp_tiles = {}
        acc_tiles = {}
        pend_hsum = []

        def flush_hsum():
            # deferred PE head-sum so PE never blocks the next exp's logits;
            # the sp tile is created here (not in attn_head) so the bufs=1
            # slot is only requested after the previous finale released it
            while pend_hsum:
                it, dg, Et, first, last = pend_hsum.pop(0)
                if it not in sp_tiles:
                    sp_tiles[it] = sp_pool.tile([128, LK], f32, tag="sp",
                                                name=f"sp{it}")
                sp = sp_tiles[it]
                for half in range(2):
                    cols = slice(half * 512, half * 512 + 512)
                    nc.tensor.matmul(
                        sp[:, cols], lhsT=dg[:], rhs=Et[:, cols],
                        start=first, stop=last,
                    )

        def attn_head(it, h):
            o, po = h // 2, (h % 2) * 64
            icols = slice(it * 128, (it + 1) * 128)
            lg = lg_pool.tile([128, LK], f32, tag="lg")
            for half in range(2):
                cols = slice(half * 512, half * 512 + 512)
                nc.tensor.matmul(
                    lg[:, cols],
                    lhsT=qT[po:po + 64, o, icols],
                    rhs=kT[po:po + 64, o, cols],
                    start=True, stop=True,
                )
            flush_hsum()
            Et = et_pool.tile([128, LK], f16, tag="et")
            rs = stats.tile([128, 1], f32, tag="rs")
            nc.scalar.activation(Et[:], lg[:], Act.Exp,
                                 scale=1.0 / np.sqrt(HD), accum_out=rs[:])
            w = stats.tile([128, 1], f32, tag="w")
            nc.vector.reciprocal(w[:], rs[:])
            if h % 2 == 0:
                # even heads: fp16 diag matmul into PSUM scores (on PE)
                dg = dg_pool.tile([128, 128], f16, tag="dg")
                nc.vector.tensor_scalar(
                    out=dg[:], in0=id16[:], scalar1=w[:], scalar2=1.0 / H,
                    op0=Alu.mult, op1=Alu.mult,
                )
                pend_hsum.append((it, dg, Et, h == 0, h == H - 2))
                if h == H - 2:
                    flush_hsum()
            else:
                # odd heads: accumulate w_h*E_h on DVE (fp16, 2x mode)
                prev = acc_tiles.get(it)
                acc = acc_pool.tile([128, LK], f16, tag="acc")
                if prev is None:
                    nc.vector.tensor_scalar(
                        out=acc[:], in0=Et[:], scalar1=w[:], scalar2=None,
                        op0=Alu.mult,
                    )
                else:
                    nc.vector.scalar_tensor_tensor(
                        out=acc[:], in0=Et[:], scalar=w[:], in1=prev[:],
                        op0=Alu.mult, op1=Alu.add,
                    )
                acc_tiles[it] = acc

        fin_state = {}

        def finale_part1(it):
            icols = slice(it * 128, (it + 1) * 128)
            sp = sp_tiles.pop(it)
            acc = acc_tiles.pop(it)
            scs = scs_pool.tile([128, LK], bf16, tag="scs")
            nc.vector.scalar_tensor_tensor(
                out=scs[:], in0=acc[:], scalar=1.0 / H, in1=sp[:],
                op0=Alu.mult, op1=Alu.add,
            )
            nc.sync.dma_start(sc[icols, :], scs[:])
            # transpose scores on the DMA xbar instead of PE
            scT = scT_pool.tile([128, 8, 128], bf16, tag="scT")
            nc.sync.dma_start_transpose(scT[:], scs[:])
            fin_state[it] = {"scT": scT, "ov": None}

        def emit_sv(it, jj):
            # one j-chunk of the attn_output matmul; jj in 0..15
            st = fin_state[it]
            icols = slice(it * 128, (it + 1) * 128)
            half, j = divmod(jj, 8)
            cols = slice(half * 512, half * 512 + 512)
            if j == 0:
                st["ov"] = tpF_pool.tile([128, 512], f32, tag="ov",
                                         name=f"ov{it}_{half}")
            nc.tensor.matmul(
                st["ov"][:], lhsT=st["scT"][:, j, :], rhs=v_sb[:, j, cols],
                start=(j == 0), stop=(j == 7),
            )
            if j == 7:
                osb = osb_pool.tile([128, 512], bf16, tag="osb")
                nc.vector.tensor_copy(osb[:], st["ov"][:])
                nc.sync.dma_start(out[icols, cols], osb[:])
                if half == 1:
                    fin_state.pop(it)

        def finale_last(it):
            # tail version: PE transposes (PE is idle at the end) pipelined
            # per-j with the attn_output matmuls, instead of waiting on the
            # high-latency DMA xbar transpose
            icols = slice(it * 128, (it + 1) * 128)
            sp = sp_tiles.pop(it)
            acc = acc_tiles.pop(it)
            scs = scs_pool.tile([128, LK], bf16, tag="scs")
            nc.vector.scalar_tensor_tensor(
                out=scs[:], in0=acc[:], scalar=1.0 / H, in1=sp[:],
                op0=Alu.mult, op1=Alu.add,
            )
            nc.sync.dma_start(sc[icols, :], scs[:])
            tpS = tpF_pool.tile([128, 8, 128], bf16, tag="ov")
            for j in range(8):
                nc.tensor.transpose(
                    tpS[:, j, :], scs[:, j * 128:(j + 1) * 128], id_sb[:])
            scT = scT_pool.tile([128, 8, 128], bf16, tag="scT")
            nc.vector.tensor_copy(scT[:], tpS[:])
            for half in range(2):
                cols = slice(half * 512, half * 512 + 512)
                ov = tpF_pool.tile([128, 512], f32, tag="ov",
                                   name=f"ovl{half}")
                for j in range(8):
                    nc.tensor.matmul(
                        ov[:], lhsT=scT[:, j, :], rhs=v_sb[:, j, cols],
                        start=(j == 0), stop=(j == 7),
                    )
                osb = osb_pool.tile([128, 512], bf16, tag="osb")
                nc.vector.tensor_copy(osb[:], ov[:])
                nc.sync.dma_start(out[icols, cols], osb[:])

        # it0 interleaved with remaining projections. Odd heads accumulate in
        # SBUF (no PSUM scores tile), so odd heads of LATER blocks are
        # "floating" work borrowed here to fill the projection-starved gaps
        # in the ACT exp stream.
        with ExitStack() as bctx:
            pools["pp"] = bctx.enter_context(
                tc.tile_pool(name="ppB", bufs=2, space="PSUM"))
            for p in range(8):
                attn_head(0, 2 * p)
                attn_head(0, 2 * p + 1)
                attn_head(1, 2 * p + 1)
                attn_head(2, 2 * p + 1)
                if p + 2 <= 7:
                    proj_o(p + 2, "k")
                    proj_o(p + 2, "q")

        load_tiled(v_sb, vv, 0, 4)
        load_tiled(v_sb, vv, 4, 8)

        with ExitStack() as fctx:
            tpF_pool = fctx.enter_context(
                tc.tile_pool(name="tpF", bufs=2, space="PSUM"))
            # later blocks run their even (PSUM) heads, interleaved with the
            # remaining floating odd heads of it3 and the previous block's
            # spread-out attn_output matmuls
            evens = list(range(2, H, 2))
            for it in range(1, 4):
                attn_head(it, 0)
                finale_part1(it - 1)
                sv_next = 0
                for i, h in enumerate(evens):
                    attn_head(it, h)
                    if it == 1:
                        attn_head(3, 2 * i + 1)
                    # scT comes from a ~5us-latency DMA transpose; start the
                    # spread attn_output matmuls once it's landed
                    if i >= 2:
                        for _ in range(3):
                            if sv_next < 16:
                                emit_sv(it - 1, sv_next)
                                sv_next += 1
                if it == 1:
                    attn_head(3, 15)
                while sv_next < 16:
                    emit_sv(it - 1, sv_next)
                    sv_next += 1
            finale_last(3)

    nc.compile()
    return nc


def _get_program():
    if "nc" not in _CACHE:
        _CACHE["nc"] = _build_program()
    return _CACHE["nc"]


def kernel(query, key, value, gq, gk, Wq, bq, Wk, bk):
    import ml_dtypes
    from concourse.bass_utils import run_bass_kernel_spmd

    nc = _get_program()
    bf16 = ml_dtypes.bfloat16

    wqt = np.ascontiguousarray((np.asarray(Wq) * np.asarray(gq)[None, :]).T
                               ).astype(bf16)
    wkt = np.ascontiguousarray((np.asarray(Wk) * np.asarray(gk)[None, :]).T
                               ).astype(bf16)
    bq2 = np.ascontiguousarray(np.asarray(bq, dtype=np.float32).reshape(8, 128).T)
    bk2 = np.ascontiguousarray(np.asarray(bk, dtype=np.float32).reshape(8, 128).T)
    ident = np.eye(128, dtype=np.float32).astype(bf16)

    q_b = np.asarray(query).astype(bf16)
    k_b = np.asarray(key).astype(bf16)
    v_b = np.asarray(value).astype(bf16)

    in_maps = []
    for c in range(N_CORES):
        b, half = divmod(c, 2)
        i0 = half * QROWS
        in_maps.append({
            "xq": np.ascontiguousarray(q_b[b, i0: i0 + QROWS]),
            "xk": np.ascontiguousarray(k_b[b]),
            "vv": np.ascontiguousarray(v_b[b]),
            "wqt": wqt, "wkt": wkt, "bq": bq2, "bk": bk2, "ident": ident,
        })

    res = run_bass_kernel_spmd(nc, in_maps, list(range(N_CORES)))

    attn_output = np.empty((B, LQ, E), dtype=np.float32)
    attn_scores = np.empty((B, LQ, LK), dtype=np.float32)
    for c in range(N_CORES):
        b, half = divmod(c, 2)
        i0 = half * QROWS
        attn_output[b, i0: i0 + QROWS] = res.results[c]["out"].astype(np.float32)
        attn_scores[b, i0: i0 + QROWS] = res.results[c]["sc"].astype(np.float32)
    return attn_output, attn_scores
